# revision 1
# baseline (speedup 1.0000x reference)
"""Trainium2 Bass kernel for nn_CLCRNModel (CLCRN encoder-decoder GNN).

Strategy: data-parallel over batch (8 batch elements -> 8 NeuronCores).
The sparse 25-neighbor graph conv is cast as dense matmuls against the
row-normalized adjacency A and its square B = A^2, both SBUF-resident in
fp8-e4m3 and streamed through the PE with DoubleRow (2 fp8 MACs/cell).

Host-side linear-algebra folds shrink every hop pass to the 64 hidden
channels:
 - encoder feature-embedding (feat = x*W_fe + b_fe) and node embedding are
   linear/constant, so their multi-hop contributions fold into precomputed
   dense groups (xab rows / nodeT rows) and biases;
 - decoder input y_t = h_t @ W_proj + b_proj exactly (autoregressive
   feedback), so the y channel folds into the h-group dense weights.
Per cell only A@h, B@h, A@(r*h), B@(r*h) are computed on the PE; A/B
scale factors (16/256, to keep fp8 in normal range) are folded into the
dense weights on the host.  Channel-major activations are bf16 so dense
matmuls stream 1024-wide and DVE elementwise runs at 16-bit rate.
"""
import os
import sys

for _p in ("/root/.axon_site/_ro/trn_rl_repo", "/opt/trn_rl_repo"):
    if os.path.isdir(_p) and _p not in sys.path:
        sys.path.append(_p)

import numpy as np
import ml_dtypes

import concourse.bass as bass
import concourse.mybir as mybir
import concourse.tile as tile
from concourse.bass_utils import run_bass_kernel_spmd
from concourse.masks import make_identity

P = 128
N = 2048
NT = 16            # node k-tiles
NPAIR = 8          # DoubleRow k-tile pairs
S = 12             # encoder steps
HOR = 12           # decoder steps
H = 64             # GRU units
FREE = 512         # hop chunk width (fp8 DR moving limit: 2x512)
NCH = N // FREE
WIDE = 1024        # dense chunk width (bf16 moving limit)
NW = N // WIDE
NCORES = 8
SA = 16.0          # fp8 scale for A
SB = 256.0         # fp8 scale for B

F32 = mybir.dt.float32
F32R = mybir.dt.float32r
BF16 = mybir.dt.bfloat16
FP8 = mybir.dt.float8e4
AF = mybir.ActivationFunctionType
DR = mybir.MatmulPerfMode.DoubleRow


def _dedup_ldweights(nc):
    """Remove Ldweights whose weights AP equals the previous PE weight
    load (PE retains the stationary operand between matmuls; walrus's own
    ldw-opt is disabled in this toolchain). Waits/updates of a removed
    load migrate to the next PE instruction."""
    import concourse.mybir as _mb
    fn = nc.m.functions[0]
    pe = _mb.EngineType.PE
    n = 0
    for blk in fn.blocks:
        out = []
        last_sig = None
        pend_waits, pend_updates = [], []
        for ins in blk.instructions:
            if ins.engine == pe:
                if ins.opcode == "Ldweights":
                    sig = str(ins.ins[0])
                    if sig == last_sig:
                        si = ins.sync_info
                        if si:
                            pend_waits.extend(si.on_wait or [])
                            pend_updates.extend(si.on_update or [])
                        n += 1
                        continue
                    last_sig = sig
                elif ins.opcode not in ("Matmult", "Drain", "EventSemaphore",
                                        "RegisterMove", "UnconditionalBranch"):
                    last_sig = None
                if pend_waits or pend_updates:
                    si = ins.sync_info
                    if si is None:
                        si = _mb.SyncInfo(on_wait=[], on_update=[])
                        ins.sync_info = si
                    si.on_wait = list(pend_waits) + list(si.on_wait or [])
                    si.on_update = list(si.on_update or []) + list(pend_updates)
                    pend_waits, pend_updates = [], []
            out.append(ins)
        assert not pend_waits and not pend_updates
        blk.instructions = out
    return n


def _split_multiwait(nc, max_waits=1):
    """This container's walrus rejects >1 sem-wait on CTRL-class
    instructions (the Tile exit drain carries one wait per live sem).
    Split excess waits onto preceding same-engine carrier drains."""
    fn = nc.m.functions[0]
    n = 0
    for blk in fn.blocks:
        out = []
        for ins in blk.instructions:
            si = ins.sync_info
            waits = list(si.on_wait) if (si and si.on_wait) else []
            if len(waits) > max_waits:
                extra, keep = waits[:-max_waits], waits[-max_waits:]
                for i in range(0, len(extra), max_waits):
                    carrier = mybir.InstDrain(
                        name=f"{ins.name}_wsplit{i}", ins=[], outs=[],
                        bass_is_fusable=False)
                    carrier.engine = ins.engine
                    carrier.sync_info = mybir.SyncInfo(
                        on_wait=extra[i:i + max_waits], on_update=[])
                    out.append(carrier)
                    n += 1
                si.on_wait = keep
            out.append(ins)
        blk.instructions = out
    return n


def _build():
    nc = bass.Bass()

    ab8_d = nc.dram_tensor("ab8", [P, NT, 2 * N], FP8, kind="ExternalInput")
    nodeT_d = nc.dram_tensor("nodeT", [48, N], BF16, kind="ExternalInput")
    xab_d = nc.dram_tensor("xab", [3, S, N], BF16, kind="ExternalInput")
    wge1_d = nc.dram_tensor("wge1", [128, 128], BF16, kind="ExternalInput")
    wge2_d = nc.dram_tensor("wge2", [115, 128], BF16, kind="ExternalInput")
    wce1_d = nc.dram_tensor("wce1", [128, 64], BF16, kind="ExternalInput")
    wce2_d = nc.dram_tensor("wce2", [115, 64], BF16, kind="ExternalInput")
    wgd01_d = nc.dram_tensor("wgd01", [128, 128], BF16, kind="ExternalInput")
    wgd02_d = nc.dram_tensor("wgd02", [64, 128], BF16, kind="ExternalInput")
    wgdf1_d = nc.dram_tensor("wgdf1", [128, 128], BF16, kind="ExternalInput")
    wgdf2_d = nc.dram_tensor("wgdf2", [64, 128], BF16, kind="ExternalInput")
    wcd01_d = nc.dram_tensor("wcd01", [128, 64], BF16, kind="ExternalInput")
    wcd02_d = nc.dram_tensor("wcd02", [64, 64], BF16, kind="ExternalInput")
    wcdy1_d = nc.dram_tensor("wcdy1", [128, 64], BF16, kind="ExternalInput")
    wcdy2_d = nc.dram_tensor("wcdy2", [64, 64], BF16, kind="ExternalInput")
    wpj_d = nc.dram_tensor("wproj", [64, 1], BF16, kind="ExternalInput")
    bias_d = nc.dram_tensor("bias", [64, 12], F32, kind="ExternalInput")
    out_d = nc.dram_tensor("out", [HOR, N], F32, kind="ExternalOutput")

    with tile.TileContext(nc) as tc:
        with tc.tile_pool(name="const", bufs=1) as cpool, \
             tc.tile_pool(name="state", bufs=1) as spool, \
             tc.tile_pool(name="psum", bufs=1, space="PSUM") as ppool:

            ab8 = cpool.tile([P, NT, 2 * N], FP8, name="ab8")
            wge1 = cpool.tile([128, 128], BF16, name="wge1")
            wge2 = cpool.tile([115, 128], BF16, name="wge2")
            wce1 = cpool.tile([128, 64], BF16, name="wce1")
            wce2 = cpool.tile([115, 64], BF16, name="wce2")
            wgd01 = cpool.tile([128, 128], BF16, name="wgd01")
            wgd02 = cpool.tile([64, 128], BF16, name="wgd02")
            wgdf1 = cpool.tile([128, 128], BF16, name="wgdf1")
            wgdf2 = cpool.tile([64, 128], BF16, name="wgdf2")
            wcd01 = cpool.tile([128, 64], BF16, name="wcd01")
            wcd02 = cpool.tile([64, 64], BF16, name="wcd02")
            wcdy1 = cpool.tile([128, 64], BF16, name="wcdy1")
            wcdy2 = cpool.tile([64, 64], BF16, name="wcdy2")
            wpj = cpool.tile([64, 1], BF16, name="wpj")
            bias = cpool.tile([64, 12], F32, name="bias")
            identb = cpool.tile([P, P], BF16, name="identb")

            h_nat = spool.tile([P, NT, H], FP8, name="h_nat")
            rh_nat = spool.tile([P, NT, H], FP8, name="rh_nat")
            # combo rhs tiles: rows 0-63 / 64-127 are separate operands so
            # one K=128 matmul covers two dense groups
            habT = spool.tile([128, N], BF16, name="habT")    # h | A@h
            bnxT = spool.tile([115, N], BF16, name="bnxT")    # B@h | node | x
            rahT = spool.tile([128, N], BF16, name="rahT")    # rh | A@rh
            brnxT = spool.tile([115, N], BF16, name="brnxT")  # B@rh | node | x
            cT = spool.tile([H, N], BF16, name="cT")
            tmpT = spool.tile([H, N], BF16, name="tmpT")
            rT = spool.tile([H, N], BF16, name="rT")
            uT = spool.tile([H, N], BF16, name="uT")
            yT = spool.tile([1, N], F32, name="yT")

            make_identity(nc, identb[:, :])

            # ---------- prologue ----------
            nc.sync.dma_start(bnxT[64:112, :], nodeT_d[:, :])
            nc.sync.dma_start(brnxT[64:112, :], nodeT_d[:, :])
            for t_sb, t_d in ((wge1, wge1_d), (wge2, wge2_d),
                              (wce1, wce1_d), (wce2, wce2_d),
                              (wgd01, wgd01_d), (wgd02, wgd02_d),
                              (wgdf1, wgdf1_d), (wgdf2, wgdf2_d),
                              (wcd01, wcd01_d), (wcd02, wcd02_d),
                              (wcdy1, wcdy1_d), (wcdy2, wcdy2_d),
                              (wpj, wpj_d), (bias, bias_d)):
                nc.sync.dma_start(t_sb[:, :], t_d[:, :])
            for k in range(NT):
                nc.sync.dma_start(ab8[:, k, 0:N], ab8_d[:, k, 0:N])
            for k in range(NT):
                nc.sync.dma_start(ab8[:, k, N:2 * N], ab8_d[:, k, N:2 * N])
            nc.vector.memset(h_nat[:, :, :], 0.0)
            nc.vector.memset(habT[:, :], 0.0)
            nc.vector.memset(rahT[:, :], 0.0)
            nc.vector.memset(bnxT[0:64, :], 0.0)
            nc.vector.memset(brnxT[0:64, :], 0.0)

            # ---------- helpers ----------
            def hop_chain(nat, dstT, row0, half, c):
                # dstT[row0:row0+H, chunk c] = ((A|B) @ z).T chunk, DR chain
                base = half * N
                hp = ppool.tile([H, FREE], F32, name="hp", tag="hp",
                                bufs=4)
                for jp in range(NPAIR):
                    nc.tensor.matmul(
                        hp[:, :],
                        nat[:, 2 * jp:2 * jp + 2, :],
                        ab8[:, 2 * jp:2 * jp + 2,
                            base + c * FREE:base + (c + 1) * FREE],
                        start=(jp == 0), stop=(jp == NPAIR - 1),
                        perf_mode=DR)
                if c != 3:
                    nc.vector.tensor_copy(
                        dstT[row0:row0 + H, c * FREE:(c + 1) * FREE],
                        hp[:, :])
                else:
                    nc.scalar.copy(
                        dstT[row0:row0 + H, c * FREE:(c + 1) * FREE],
                        hp[:, :])

            def hop(nat, dstT, row0, half):
                for c in range(NCH):
                    hop_chain(nat, dstT, row0, half, c)

            def mm_groups(dp, m, groups, c):
                ng = len(groups)
                for gi, (w_ap, rhs, kr) in enumerate(groups):
                    nc.tensor.matmul(
                        dp[0:m, :], w_ap,
                        rhs[0:kr, c * FREE:(c + 1) * FREE],
                        start=(gi == 0), stop=(gi == ng - 1))

            def gate_dense_chunk(groups, rcol, ucol, c):
                # fused r|u: psum rows 0-63 -> rT, 64-127 -> uT
                dp = ppool.tile([P, FREE], F32, name="dp", tag="dp",
                                bufs=2)
                mm_groups(dp, 128, groups, c)
                sl = slice(c * FREE, (c + 1) * FREE)
                nc.scalar.activation(rT[:, sl], dp[0:64, :], AF.Sigmoid,
                                     bias=bias[:, rcol:rcol + 1])
                nc.scalar.activation(uT[:, sl], dp[64:128, :], AF.Sigmoid,
                                     bias=bias[:, ucol:ucol + 1])

            def gate_dense(groups, rcol, ucol):
                for c in range(NCH):
                    gate_dense_chunk(groups, rcol, ucol, c)

            def cand_dense_chunk(groups, bcol, c):
                dp = ppool.tile([P, FREE], F32, name="dp", tag="dp",
                                bufs=2)
                mm_groups(dp, 64, groups, c)
                sl = slice(c * FREE, (c + 1) * FREE)
                nc.scalar.activation(cT[:, sl], dp[0:64, :], AF.Tanh,
                                     bias=bias[:, bcol:bcol + 1])

            def to_nat_group(srcT, dst, g):
                # natural fp8 tiles for 4 k-tiles: 4 transposes batched per
                # psum tile, one cast copy
                j0 = 4 * g
                tp = ppool.tile([P, 4 * H], BF16, name="tp", tag="tp",
                                bufs=2)
                for jj in range(4):
                    nc.tensor.transpose(
                        tp[:, jj * H:(jj + 1) * H],
                        srcT[0:H, (j0 + jj) * P:(j0 + jj + 1) * P],
                        identb[0:H, 0:H])
                if g % 2 == 1:
                    nc.vector.tensor_copy(dst[:, j0:j0 + 4, :], tp[:, :])
                else:
                    nc.scalar.copy(dst[:, j0:j0 + 4, :], tp[:, :])

            def update_chunk(c, last):
                # h' = c + u*(h-c)
                sl = slice(c * FREE, (c + 1) * FREE)
                nc.vector.tensor_sub(tmpT[:, sl], habT[0:H, sl], cT[:, sl])
                nc.vector.tensor_mul(tmpT[:, sl], tmpT[:, sl], uT[:, sl])
                nc.vector.tensor_add(habT[0:H, sl], tmpT[:, sl], cT[:, sl])
                if not last:
                    to_nat_group(habT, h_nat, c)

            def make_rh_chunk(c):
                sl = slice(c * FREE, (c + 1) * FREE)
                nc.vector.tensor_mul(rahT[0:H, sl], rT[:, sl],
                                     habT[0:H, sl])
                to_nat_group(rahT, rh_nat, c)

            # ---------- encoder ----------
            for t in range(S):
                nc.sync.dma_start(bnxT[112:115, :], xab_d[:, t, :])
                nc.sync.dma_start(brnxT[112:115, :], xab_d[:, t, :])
                have_h = t > 0
                # t=0: h == 0 exactly, so the habT/rahT groups would only
                # accumulate +0.0 -- drop them (bit-identical)
                g_groups = [(wge2[:, :], bnxT, 115)]
                c_groups = [(wce2[:, :], brnxT, 115)]
                if have_h:
                    g_groups = [(wge1[:, :], habT, 128)] + g_groups
                    c_groups = [(wce1[:, :], rahT, 128)] + c_groups
                for c in range(NCH):
                    if have_h:
                        hop_chain(h_nat, habT, H, 0, c)
                        hop_chain(h_nat, bnxT, 0, 1, c)
                    if c > 0:
                        gate_dense_chunk(g_groups, 0, 1, c - 1)
                        if have_h:
                            make_rh_chunk(c - 1)
                gate_dense_chunk(g_groups, 0, 1, NCH - 1)
                if have_h:
                    make_rh_chunk(NCH - 1)
                for c in range(NCH):
                    if have_h:
                        hop_chain(rh_nat, rahT, H, 0, c)
                        hop_chain(rh_nat, brnxT, 0, 1, c)
                    if c > 0:
                        cand_dense_chunk(c_groups, 2, c - 1)
                        update_chunk(c - 1, last=False)
                cand_dense_chunk(c_groups, 2, NCH - 1)
                update_chunk(NCH - 1, last=False)

            # ---------- decoder ----------
            for u in range(HOR):
                wg1, wg2 = (wgd01, wgd02) if u == 0 else (wgdf1, wgdf2)
                g_groups = [(wg1[:, :], habT, 128), (wg2[:, :], bnxT, 64)]
                c_groups = [(wcd01[:, :], rahT, 128),
                            (wcd02[:, :], brnxT, 64)]
                if u > 0:
                    c_groups = [(wcdy1[:, :], habT, 128),
                                (wcdy2[:, :], bnxT, 64)] + c_groups
                rc, uc = (3, 4) if u == 0 else (5, 6)
                for c in range(NCH):
                    hop_chain(h_nat, habT, H, 0, c)
                    hop_chain(h_nat, bnxT, 0, 1, c)
                    if c > 0:
                        gate_dense_chunk(g_groups, rc, uc, c - 1)
                        make_rh_chunk(c - 1)
                gate_dense_chunk(g_groups, rc, uc, NCH - 1)
                make_rh_chunk(NCH - 1)
                bc = 7 if u == 0 else 8
                for c in range(NCH):
                    hop_chain(rh_nat, rahT, H, 0, c)
                    hop_chain(rh_nat, brnxT, 0, 1, c)
                    if c > 0:
                        cand_dense_chunk(c_groups, bc, c - 1)
                        update_chunk(c - 1, last=(u == HOR - 1))
                cand_dense_chunk(c_groups, bc, NCH - 1)
                update_chunk(NCH - 1, last=(u == HOR - 1))
                # y = h' @ Wproj + b  (output only; feedback is folded)
                for c in range(NCH):
                    yp = ppool.tile([P, FREE], F32, name="yp", tag="dp",
                                    bufs=2)
                    nc.tensor.matmul(yp[0:1, :], wpj[:, :],
                                     habT[0:H, c * FREE:(c + 1) * FREE],
                                     start=True, stop=True)
                    nc.scalar.activation(yT[0:1, c * FREE:(c + 1) * FREE],
                                         yp[0:1, :], AF.Identity,
                                         bias=bias[0:1, 9:10])
                nc.sync.dma_start(out_d[u:u + 1, :], yT[:, :])

    _dedup_ldweights(nc)
    _split_multiwait(nc)
    return nc


# ---------------- host-side preprocessing ----------------

def _softplus(x):
    return np.log1p(np.exp(-np.abs(x))) + np.maximum(x, 0.0)


def _q8(x):
    # TRN e4m3 overflows to inf above +-240 (unlike OCP's 448): clip first.
    return np.clip(np.asarray(x, np.float32), -240.0, 240.0).astype(
        ml_dtypes.float8_e4m3)


def _host_prep(inp):
    """Edge-weight MLP + row-normalization + dense A, B=A^2 build + all
    linearity folds. Pure per-graph preprocessing (no time loop)."""
    f = np.float32
    bf = ml_dtypes.bfloat16
    row, col = np.asarray(inp["sparse_idx"])
    loc = np.asarray(inp["loc"], f)
    delta = loc[col] - loc[row]
    h1 = np.tanh(delta @ np.asarray(inp["Wk0"], f) + np.asarray(inp["bk0"], f))
    h2 = np.tanh(h1 @ np.asarray(inp["Wk1"], f) + np.asarray(inp["bk1"], f))
    ker = _softplus((h2 @ np.asarray(inp["Wk2"], f)
                     + np.asarray(inp["bk2"], f))[:, 0])
    geo = np.asarray(inp["geodesic"], f)
    w = ker * np.asarray(inp["angle_ratio"], f) * np.exp(-geo * geo)
    denom = np.zeros(N, f)
    np.add.at(denom, row, w)
    w = (w / (denom[row] + np.float32(1e-8))).astype(f)
    A = np.zeros((N, N), f)
    np.add.at(A, (row, col), w)
    B = A @ A

    # fp8 A/B, transposed+tiled for the moving operand:
    # ab8[p, j, half*N + m] = M[m, j*128 + p], M in {A*SA, B*SB}
    a8 = _q8(A.T * SA).reshape(NT, P, N).transpose(1, 0, 2)
    b8 = _q8(B.T * SB).reshape(NT, P, N).transpose(1, 0, 2)
    ab8 = np.concatenate([a8, b8], axis=2)
    a8f = a8.astype(f)
    b8f = b8.astype(f)

    Wfe = np.asarray(inp["W_fe"], f)      # (1, 16)
    bfe = np.asarray(inp["b_fe"], f)
    Wp = np.asarray(inp["W_proj"], f)     # (64, 1)
    bp = np.asarray(inp["b_proj"], f)
    node = np.asarray(inp["node_emb"], f)
    SC = [1.0, SA, SB]

    # encoder fold: z rows per hop k are [feat16 | node16 | x1 | h64]
    def enc_fold(W):
        out = W.shape[1]
        Wx = np.zeros((3, out), f)
        b_extra = np.zeros(out, f)
        Wh = np.zeros((64, 3 * out), f)
        for k in range(3):
            Wk = W[k * 97:(k + 1) * 97]
            Wf, Wxr, Whk = Wk[0:16], Wk[32:33], Wk[33:97]
            Wx[k] = (Wxr[0] + Wfe[0] @ Wf) / SC[k]
            b_extra += bfe @ Wf
            Wh[:, k * out:(k + 1) * out] = Whk / SC[k]
        return Wx, Wh, b_extra

    Wg_e = np.asarray(inp["Wg_e"], f)
    Wc_e = np.asarray(inp["Wc_e"], f)
    wgx, wge, bg_x = enc_fold(Wg_e)
    wcx, wce, bc_x = enc_fold(Wc_e)
    bg_e = np.asarray(inp["bg_e"], f) + bg_x
    bc_e = np.asarray(inp["bc_e"], f) + bc_x

    # node rhs rows: [node.T; (A node).T; (B node).T] with per-hop weight
    # blocks stacked in wgn/wcn rows 0-47 (exact f32 A/B on host); rows
    # 48-50 hold the folded x/Ax/Bx weights (rhs rows DMA'd per step)
    nodeT = np.concatenate([node.T, (A @ node).T, (B @ node).T], axis=0)
    wgn = np.zeros((51, 128), f)
    wcn = np.zeros((51, 64), f)
    for k in range(3):
        wgn[k * 16:(k + 1) * 16] = Wg_e[k * 97 + 16:k * 97 + 32]
        wcn[k * 16:(k + 1) * 16] = Wc_e[k * 97 + 16:k * 97 + 32]
    wgn[48:51] = wgx
    wcn[48:51] = wcx

    # decoder fold: z rows per hop k are [y1 | h64]
    Wg_d = np.asarray(inp["Wg_d"], f)
    Wc_d = np.asarray(inp["Wc_d"], f)

    def dec_fold(W):
        out = W.shape[1]
        Wh_plain = np.zeros((64, 3 * out), f)
        Wh_fold = np.zeros((64, 3 * out), f)
        Wy_h = np.zeros((64, 3 * out), f)
        b_extra = np.zeros(out, f)
        for k in range(3):
            Wk = W[k * 65:(k + 1) * 65]
            Wy, Wh = Wk[0:1], Wk[1:65]
            Wh_plain[:, k * out:(k + 1) * out] = Wh / SC[k]
            Wh_fold[:, k * out:(k + 1) * out] = (Wh + Wp @ Wy) / SC[k]
            Wy_h[:, k * out:(k + 1) * out] = (Wp @ Wy) / SC[k]
            b_extra += bp @ Wy
        return Wh_plain, Wh_fold, Wy_h, b_extra

    wgd0, wgdf, _, bgd_x = dec_fold(Wg_d)
    wcd0, _, wcdy, bcd_x = dec_fold(Wc_d)
    bg_d = np.asarray(inp["bg_d"], f)
    bc_d = np.asarray(inp["bc_d"], f)

    bias = np.zeros((64, 12), f)
    bias[:, 0] = bg_e[0:64]
    bias[:, 1] = bg_e[64:128]
    bias[:, 2] = bc_e
    bias[:, 3] = bg_d[0:64]
    bias[:, 4] = bg_d[64:128]
    bias[:, 5] = (bg_d + bgd_x)[0:64]
    bias[:, 6] = (bg_d + bgd_x)[64:128]
    bias[:, 7] = bc_d
    bias[:, 8] = bc_d + bcd_x
    bias[0, 9] = bp[0]

    shared = {
        "ab8": ab8,
        "nodeT": np.ascontiguousarray(nodeT).astype(bf),
        "wge1": np.concatenate([wge[:, 0:128], wge[:, 128:256]]).astype(bf),
        "wge2": np.concatenate([wge[:, 256:384], wgn]).astype(bf),
        "wce1": np.concatenate([wce[:, 0:64], wce[:, 64:128]]).astype(bf),
        "wce2": np.concatenate([wce[:, 128:192], wcn]).astype(bf),
        "wgd01": np.concatenate([wgd0[:, 0:128], wgd0[:, 128:256]]).astype(bf),
        "wgd02": wgd0[:, 256:384].astype(bf),
        "wgdf1": np.concatenate([wgdf[:, 0:128], wgdf[:, 128:256]]).astype(bf),
        "wgdf2": wgdf[:, 256:384].astype(bf),
        "wcd01": np.concatenate([wcd0[:, 0:64], wcd0[:, 64:128]]).astype(bf),
        "wcd02": wcd0[:, 128:192].astype(bf),
        "wcdy1": np.concatenate([wcdy[:, 0:64], wcdy[:, 64:128]]).astype(bf),
        "wcdy2": wcdy[:, 128:192].astype(bf),
        "wproj": Wp.astype(bf), "bias": bias,
    }

    xs = np.asarray(inp["inputs"], f)[:, :, :, 0]    # (S, B, N)
    a8m = a8f.transpose(1, 0, 2).reshape(N, N)       # a8m[n_in, m] = A8[m, n_in]
    b8m = b8f.transpose(1, 0, 2).reshape(N, N)
    in_maps = []
    for b in range(NCORES):
        X = xs[:, b, :]                              # (S, N)
        Xq = _q8(X).astype(f)
        # xab rows per step t: [x_t; (A8 @ q8(x_t)).T; (B8 @ q8(x_t)).T]
        AXt = Xq @ a8m                               # (S, N)
        BXt = Xq @ b8m
        xab = np.stack([X, AXt, BXt])                # (3, S, N)
        m = dict(shared)
        m["xab"] = np.ascontiguousarray(xab).astype(bf)
        in_maps.append(m)
    return in_maps


_NC_CACHE = []


def kernel(**inputs):
    if not _NC_CACHE:
        _NC_CACHE.append(_build())
    nc = _NC_CACHE[0]
    in_maps = _host_prep(inputs)
    res = run_bass_kernel_spmd(nc, in_maps, core_ids=list(range(NCORES)))
    out = np.stack([res.results[b]["out"] for b in range(NCORES)], axis=1)
    return np.ascontiguousarray(out[..., None].astype(np.float32))



# revision 2
# speedup vs baseline: 2.2147x; 2.2147x over previous
"""Trainium2 Bass kernel for nn_CLCRNModel (CLCRN encoder-decoder GNN).

Strategy: data-parallel over batch (8 batch elements -> 8 NeuronCores).
The sparse 25-neighbor graph conv is cast as dense matmuls against the
row-normalized adjacency A and its square B = A^2, both SBUF-resident in
fp8-e4m3 and streamed through the PE with DoubleRow (2 fp8 MACs/cell).

Input-volume optimization: every core receives only a 2-k-tile slice of
A (in both moving/T and stationary/natural orientations, 1 MB); the full
fp8 A is assembled on device with one HBM AllGather, and B = A@A is
computed on the PE (fp8 DR) directly into the SBUF hop operand, so no
dense matrix ever crosses the host-device link.

Host-side linear-algebra folds shrink every hop pass to the 64 hidden
channels:
 - encoder feature-embedding (feat = x*W_fe + b_fe) and node embedding are
   linear/constant, so their multi-hop contributions fold into precomputed
   dense groups (xab rows / nodeT rows) and biases;
 - decoder input y_t = h_t @ W_proj + b_proj exactly (autoregressive
   feedback), so the y channel folds into the h-group dense weights.
Per cell only A@h, B@h, A@(r*h), B@(r*h) are computed on the PE; A/B
scale factors (16/128, to keep fp8 in normal range) are folded into the
dense weights on the host.  Channel-major activations are bf16 so dense
matmuls stream 1024-wide and DVE elementwise runs at 16-bit rate.
"""
import os
import sys

for _p in ("/root/.axon_site/_ro/trn_rl_repo", "/opt/trn_rl_repo"):
    if os.path.isdir(_p) and _p not in sys.path:
        sys.path.append(_p)

import numpy as np
import ml_dtypes

import concourse.bass as bass
import concourse.mybir as mybir
import concourse.tile as tile
from concourse.bass_utils import run_bass_kernel_spmd
from concourse.masks import make_identity

P = 128
N = 2048
NT = 16            # node k-tiles
NPAIR = 8          # DoubleRow k-tile pairs
S = 12             # encoder steps
HOR = 12           # decoder steps
H = 64             # GRU units
FREE = 512         # hop chunk width (fp8 DR moving limit: 2x512)
NCH = N // FREE
NCORES = 8
KSL = NT // NCORES  # k-tiles per core slice
SA = 16.0          # fp8 scale for A
SB = 128.0         # fp8 scale for B (= SA^2 * 0.5, applied in B-build copy)

F32 = mybir.dt.float32
F32R = mybir.dt.float32r
BF16 = mybir.dt.bfloat16
FP8 = mybir.dt.float8e4
I16 = mybir.dt.int16
AF = mybir.ActivationFunctionType
DR = mybir.MatmulPerfMode.DoubleRow


def _dedup_ldweights(nc):
    """Remove Ldweights whose weights AP equals the previous PE weight
    load (PE retains the stationary operand between matmuls; walrus's own
    ldw-opt is disabled in this toolchain). Waits/updates of a removed
    load migrate to the next PE instruction."""
    import concourse.mybir as _mb
    fn = nc.m.functions[0]
    pe = _mb.EngineType.PE
    n = 0
    for blk in fn.blocks:
        out = []
        last_sig = None
        pend_waits, pend_updates = [], []
        for ins in blk.instructions:
            if ins.engine == pe:
                if ins.opcode == "Ldweights":
                    sig = str(ins.ins[0])
                    if sig == last_sig:
                        si = ins.sync_info
                        if si:
                            pend_waits.extend(si.on_wait or [])
                            pend_updates.extend(si.on_update or [])
                        n += 1
                        continue
                    last_sig = sig
                elif ins.opcode not in ("Matmult", "Drain", "EventSemaphore",
                                        "RegisterMove", "UnconditionalBranch"):
                    last_sig = None
                if pend_waits or pend_updates:
                    si = ins.sync_info
                    if si is None:
                        si = _mb.SyncInfo(on_wait=[], on_update=[])
                        ins.sync_info = si
                    si.on_wait = list(pend_waits) + list(si.on_wait or [])
                    si.on_update = list(si.on_update or []) + list(pend_updates)
                    pend_waits, pend_updates = [], []
            out.append(ins)
        assert not pend_waits and not pend_updates
        blk.instructions = out
    return n


def _split_multiwait(nc, max_waits=1):
    """This container's walrus rejects >1 sem-wait on CTRL-class
    instructions (the Tile exit drain carries one wait per live sem).
    Split excess waits onto preceding same-engine carrier drains."""
    fn = nc.m.functions[0]
    n = 0
    for blk in fn.blocks:
        out = []
        for ins in blk.instructions:
            si = ins.sync_info
            waits = list(si.on_wait) if (si and si.on_wait) else []
            if len(waits) > max_waits:
                extra, keep = waits[:-max_waits], waits[-max_waits:]
                for i in range(0, len(extra), max_waits):
                    carrier = mybir.InstDrain(
                        name=f"{ins.name}_wsplit{i}", ins=[], outs=[],
                        bass_is_fusable=False)
                    carrier.engine = ins.engine
                    carrier.sync_info = mybir.SyncInfo(
                        on_wait=extra[i:i + max_waits], on_update=[])
                    out.append(carrier)
                    n += 1
                si.on_wait = keep
            out.append(ins)
        blk.instructions = out
    return n


def _build(dist=True):
    nc = bass.Bass(num_devices=NCORES) if dist else bass.Bass()

    if dist:
        aab_d = nc.dram_tensor("aab", [2 * KSL, P, N], FP8,
                               kind="ExternalInput")
        aab_i = nc.dram_tensor("aab_i", [2 * KSL, P, N], FP8,
                               kind="Internal")
        aab_g = nc.dram_tensor("aab_g", [2 * KSL * NCORES, P, N], FP8,
                               kind="Internal", addr_space="Shared")
    else:
        afull_d = nc.dram_tensor("afull", [NT, P, N], FP8,
                                 kind="ExternalInput")
        anfull_d = nc.dram_tensor("anfull", [NT, P, N], FP8,
                                  kind="ExternalInput")
    nodeT_d = nc.dram_tensor("nodeT", [48, N], BF16, kind="ExternalInput")
    xab_d = nc.dram_tensor("xab", [3, S, N], BF16, kind="ExternalInput")
    wge1_d = nc.dram_tensor("wge1", [128, 128], BF16, kind="ExternalInput")
    wge2_d = nc.dram_tensor("wge2", [115, 128], BF16, kind="ExternalInput")
    wce1_d = nc.dram_tensor("wce1", [128, 64], BF16, kind="ExternalInput")
    wce2_d = nc.dram_tensor("wce2", [115, 64], BF16, kind="ExternalInput")
    wgd01_d = nc.dram_tensor("wgd01", [128, 128], BF16, kind="ExternalInput")
    wgd02_d = nc.dram_tensor("wgd02", [64, 128], BF16, kind="ExternalInput")
    wgdf1_d = nc.dram_tensor("wgdf1", [128, 128], BF16, kind="ExternalInput")
    wgdf2_d = nc.dram_tensor("wgdf2", [64, 128], BF16, kind="ExternalInput")
    wcd01_d = nc.dram_tensor("wcd01", [128, 64], BF16, kind="ExternalInput")
    wcd02_d = nc.dram_tensor("wcd02", [64, 64], BF16, kind="ExternalInput")
    wcdy1_d = nc.dram_tensor("wcdy1", [128, 64], BF16, kind="ExternalInput")
    wcdy2_d = nc.dram_tensor("wcdy2", [64, 64], BF16, kind="ExternalInput")
    wpj_d = nc.dram_tensor("wproj", [64, 1], BF16, kind="ExternalInput")
    bias_d = nc.dram_tensor("bias", [64, 12], F32, kind="ExternalInput")
    out_d = nc.dram_tensor("out", [HOR, N], F32, kind="ExternalOutput")

    with tile.TileContext(nc) as tc:
        with tc.tile_pool(name="const", bufs=1) as cpool, \
             tc.tile_pool(name="state", bufs=1) as spool, \
             tc.tile_pool(name="psum", bufs=1, space="PSUM") as ppool:

            ab8 = cpool.tile([P, NT, 2 * N], FP8, name="ab8")
            anat = cpool.tile([P, NT, N], FP8, name="anat")
            wge1 = cpool.tile([128, 128], BF16, name="wge1")
            wge2 = cpool.tile([115, 128], BF16, name="wge2")
            wce1 = cpool.tile([128, 64], BF16, name="wce1")
            wce2 = cpool.tile([115, 64], BF16, name="wce2")
            wgd01 = cpool.tile([128, 128], BF16, name="wgd01")
            wgd02 = cpool.tile([64, 128], BF16, name="wgd02")
            wgdf1 = cpool.tile([128, 128], BF16, name="wgdf1")
            wgdf2 = cpool.tile([64, 128], BF16, name="wgdf2")
            wcd01 = cpool.tile([128, 64], BF16, name="wcd01")
            wcd02 = cpool.tile([64, 64], BF16, name="wcd02")
            wcdy1 = cpool.tile([128, 64], BF16, name="wcdy1")
            wcdy2 = cpool.tile([64, 64], BF16, name="wcdy2")
            wpj = cpool.tile([64, 1], BF16, name="wpj")
            bias = cpool.tile([64, 12], F32, name="bias")
            identb = cpool.tile([P, P], BF16, name="identb")

            h_nat = spool.tile([P, NT, H], FP8, name="h_nat")
            rh_nat = spool.tile([P, NT, H], FP8, name="rh_nat")
            # combo rhs tiles: rows 0-63 / 64-127 are separate operands so
            # one K=128 matmul covers two dense groups
            habT = spool.tile([128, N], BF16, name="habT")    # h | A@h
            bnxT = spool.tile([115, N], BF16, name="bnxT")    # B@h | node | x
            rahT = spool.tile([128, N], BF16, name="rahT")    # rh | A@rh
            brnxT = spool.tile([115, N], BF16, name="brnxT")  # B@rh | node | x
            cT = spool.tile([H, N], BF16, name="cT")
            tmpT = spool.tile([H, N], BF16, name="tmpT")
            rT = spool.tile([H, N], BF16, name="rT")
            uT = spool.tile([H, N], BF16, name="uT")
            yT = spool.tile([1, N], F32, name="yT")

            make_identity(nc, identb[:, :])

            # ---------- prologue ----------
            nc.sync.dma_start(bnxT[64:112, :], nodeT_d[:, :])
            nc.sync.dma_start(brnxT[64:112, :], nodeT_d[:, :])
            for t_sb, t_d in ((wge1, wge1_d), (wge2, wge2_d),
                              (wce1, wce1_d), (wce2, wce2_d),
                              (wgd01, wgd01_d), (wgd02, wgd02_d),
                              (wgdf1, wgdf1_d), (wgdf2, wgdf2_d),
                              (wcd01, wcd01_d), (wcd02, wcd02_d),
                              (wcdy1, wcdy1_d), (wcdy2, wcdy2_d),
                              (wpj, wpj_d), (bias, bias_d)):
                nc.sync.dma_start(t_sb[:, :], t_d[:, :])
            if dist:
                nc.sync.dma_start(aab_i[:, :, :], aab_d[:, :, :])
                nc.gpsimd.collective_compute(
                    "AllGather", mybir.AluOpType.bypass,
                    replica_groups=[list(range(NCORES))],
                    ins=[aab_i[:, :, :]], outs=[aab_g[:, :, :]])
                for k in range(NT):
                    blk = 2 * KSL * (k // KSL) + (k % KSL)
                    nc.sync.dma_start(ab8[:, k, 0:N], aab_g[blk, :, :])
                for j in range(NT):
                    blk = 2 * KSL * (j // KSL) + KSL + (j % KSL)
                    nc.sync.dma_start(anat[:, j, :], aab_g[blk, :, :])
            else:
                for k in range(NT):
                    nc.sync.dma_start(ab8[:, k, 0:N], afull_d[k, :, :])
                for j in range(NT):
                    nc.sync.dma_start(anat[:, j, :], anfull_d[j, :, :])
            nc.vector.memset(h_nat[:, :, :], 0.0)
            nc.vector.memset(habT[:, :], 0.0)
            nc.vector.memset(rahT[:, :], 0.0)
            nc.vector.memset(bnxT[0:64, :], 0.0)
            nc.vector.memset(brnxT[0:64, :], 0.0)

            # ---------- helpers ----------
            def emit_b_build():
                # B^T tiles = (T @ T) with T = A^T: stationary = natural
                # A row-tiles, moving = T tiles; out_bp[p, f] =
                # SA^2 * B^T[t*128+p, c*512+f]; stored *SB/SA^2 as fp8.
                for t in range(NT):
                    bp = [ppool.tile([P, FREE], F32, name=f"bp{c}",
                                     tag="hp", bufs=4) for c in range(NCH)]
                    for l in range(NPAIR):
                        for c in range(NCH):
                            nc.tensor.matmul(
                                bp[c][:, :],
                                anat[:, 2 * l:2 * l + 2, t * P:(t + 1) * P],
                                ab8[:, 2 * l:2 * l + 2,
                                    c * FREE:(c + 1) * FREE],
                                start=(l == 0), stop=(l == NPAIR - 1),
                                perf_mode=DR)
                    for c in range(NCH):
                        nc.vector.tensor_scalar_mul(
                            ab8[:, t, N + c * FREE:N + (c + 1) * FREE],
                            bp[c][:, :], SB / (SA * SA))

            def hop_chain(nat, dstT, row0, half, c):
                # dstT[row0:row0+H, chunk c] = ((A|B) @ z).T chunk, DR chain
                base = half * N
                hp = ppool.tile([P, FREE], F32, name="hp", tag="hp",
                                bufs=4)
                for jp in range(NPAIR):
                    nc.tensor.matmul(
                        hp[0:H, :],
                        nat[:, 2 * jp:2 * jp + 2, :],
                        ab8[:, 2 * jp:2 * jp + 2,
                            base + c * FREE:base + (c + 1) * FREE],
                        start=(jp == 0), stop=(jp == NPAIR - 1),
                        perf_mode=DR)
                if c != 3:
                    nc.vector.tensor_copy(
                        dstT[row0:row0 + H, c * FREE:(c + 1) * FREE],
                        hp[0:H, :])
                else:
                    nc.scalar.copy(
                        dstT[row0:row0 + H, c * FREE:(c + 1) * FREE],
                        hp[0:H, :])

            def mm_groups(dp, m, groups, c):
                ng = len(groups)
                for gi, (w_ap, rhs, kr) in enumerate(groups):
                    nc.tensor.matmul(
                        dp[0:m, :], w_ap,
                        rhs[0:kr, c * FREE:(c + 1) * FREE],
                        start=(gi == 0), stop=(gi == ng - 1))

            def gate_dense_chunk(groups, rcol, ucol, c):
                # fused r|u: psum rows 0-63 -> rT, 64-127 -> uT
                dp = ppool.tile([P, FREE], F32, name="dp", tag="dp",
                                bufs=2)
                mm_groups(dp, 128, groups, c)
                sl = slice(c * FREE, (c + 1) * FREE)
                nc.scalar.activation(rT[:, sl], dp[0:64, :], AF.Sigmoid,
                                     bias=bias[:, rcol:rcol + 1])
                nc.scalar.activation(uT[:, sl], dp[64:128, :], AF.Sigmoid,
                                     bias=bias[:, ucol:ucol + 1])

            def gate_dense(groups, rcol, ucol):
                for c in range(NCH):
                    gate_dense_chunk(groups, rcol, ucol, c)

            def cand_dense_chunk(groups, bcol, c):
                dp = ppool.tile([P, FREE], F32, name="dp", tag="dp",
                                bufs=2)
                mm_groups(dp, 64, groups, c)
                sl = slice(c * FREE, (c + 1) * FREE)
                nc.scalar.activation(cT[:, sl], dp[0:64, :], AF.Tanh,
                                     bias=bias[:, bcol:bcol + 1])

            def to_nat_group(srcT, dst, g):
                # natural fp8 tiles for 4 k-tiles: 4 transposes batched per
                # psum tile, one cast copy
                j0 = 4 * g
                tp = ppool.tile([P, 4 * H], BF16, name="tp", tag="tp",
                                bufs=2)
                for jj in range(4):
                    nc.tensor.transpose(
                        tp[:, jj * H:(jj + 1) * H],
                        srcT[0:H, (j0 + jj) * P:(j0 + jj + 1) * P],
                        identb[0:H, 0:H])
                if g % 2 == 1:
                    nc.vector.tensor_copy(dst[:, j0:j0 + 4, :], tp[:, :])
                else:
                    nc.scalar.copy(dst[:, j0:j0 + 4, :], tp[:, :])

            def update_chunk(c, last):
                # h' = c + u*(h-c)
                sl = slice(c * FREE, (c + 1) * FREE)
                nc.vector.tensor_sub(tmpT[:, sl], habT[0:H, sl], cT[:, sl])
                nc.vector.tensor_mul(tmpT[:, sl], tmpT[:, sl], uT[:, sl])
                nc.vector.tensor_add(habT[0:H, sl], tmpT[:, sl], cT[:, sl])
                if not last:
                    to_nat_group(habT, h_nat, c)

            def make_rh_chunk(c):
                sl = slice(c * FREE, (c + 1) * FREE)
                nc.vector.tensor_mul(rahT[0:H, sl], rT[:, sl],
                                     habT[0:H, sl])
                to_nat_group(rahT, rh_nat, c)

            # ---------- encoder ----------
            def enc_step(t):
                nc.sync.dma_start(bnxT[112:115, :], xab_d[:, t, :])
                nc.sync.dma_start(brnxT[112:115, :], xab_d[:, t, :])
                have_h = t > 0
                # t=0: h == 0 exactly, so the habT/rahT groups would only
                # accumulate +0.0 -- drop them (bit-identical)
                g_groups = [(wge2[:, :], bnxT, 115)]
                c_groups = [(wce2[:, :], brnxT, 115)]
                if have_h:
                    g_groups = [(wge1[:, :], habT, 128)] + g_groups
                    c_groups = [(wce1[:, :], rahT, 128)] + c_groups
                for c in range(NCH):
                    if have_h:
                        hop_chain(h_nat, habT, H, 0, c)
                        hop_chain(h_nat, bnxT, 0, 1, c)
                    if c > 0:
                        gate_dense_chunk(g_groups, 0, 1, c - 1)
                        if have_h:
                            make_rh_chunk(c - 1)
                gate_dense_chunk(g_groups, 0, 1, NCH - 1)
                if have_h:
                    make_rh_chunk(NCH - 1)
                for c in range(NCH):
                    if have_h:
                        hop_chain(rh_nat, rahT, H, 0, c)
                        hop_chain(rh_nat, brnxT, 0, 1, c)
                    if c > 0:
                        cand_dense_chunk(c_groups, 2, c - 1)
                        update_chunk(c - 1, last=False)
                cand_dense_chunk(c_groups, 2, NCH - 1)
                update_chunk(NCH - 1, last=False)

            enc_step(0)
            # B-build is emitted after the (hop-free) t=0 cell so the PE
            # works on t=0's dense matmuls while the A AllGather completes.
            emit_b_build()
            for t in range(1, S):
                enc_step(t)

            # ---------- decoder ----------
            for u in range(HOR):
                wg1, wg2 = (wgd01, wgd02) if u == 0 else (wgdf1, wgdf2)
                g_groups = [(wg1[:, :], habT, 128), (wg2[:, :], bnxT, 64)]
                c_groups = [(wcd01[:, :], rahT, 128),
                            (wcd02[:, :], brnxT, 64)]
                if u > 0:
                    c_groups = [(wcdy1[:, :], habT, 128),
                                (wcdy2[:, :], bnxT, 64)] + c_groups
                rc, uc = (3, 4) if u == 0 else (5, 6)
                for c in range(NCH):
                    hop_chain(h_nat, habT, H, 0, c)
                    hop_chain(h_nat, bnxT, 0, 1, c)
                    if c > 0:
                        gate_dense_chunk(g_groups, rc, uc, c - 1)
                        make_rh_chunk(c - 1)
                gate_dense_chunk(g_groups, rc, uc, NCH - 1)
                make_rh_chunk(NCH - 1)
                bc = 7 if u == 0 else 8
                for c in range(NCH):
                    hop_chain(rh_nat, rahT, H, 0, c)
                    hop_chain(rh_nat, brnxT, 0, 1, c)
                    if c > 0:
                        cand_dense_chunk(c_groups, bc, c - 1)
                        update_chunk(c - 1, last=(u == HOR - 1))
                cand_dense_chunk(c_groups, bc, NCH - 1)
                update_chunk(NCH - 1, last=(u == HOR - 1))
                # y = h' @ Wproj + b  (output only; feedback is folded)
                for c in range(NCH):
                    yp = ppool.tile([P, FREE], F32, name="yp", tag="dp",
                                    bufs=2)
                    nc.tensor.matmul(yp[0:1, :], wpj[:, :],
                                     habT[0:H, c * FREE:(c + 1) * FREE],
                                     start=True, stop=True)
                    nc.scalar.activation(yT[0:1, c * FREE:(c + 1) * FREE],
                                         yp[0:1, :], AF.Identity,
                                         bias=bias[0:1, 9:10])
                nc.sync.dma_start(out_d[u:u + 1, :], yT[:, :])

    _dedup_ldweights(nc)
    _split_multiwait(nc)
    return nc


# ---------------- host-side preprocessing ----------------

def _softplus(x):
    return np.log1p(np.exp(-np.abs(x))) + np.maximum(x, 0.0)


def _q8(x):
    # TRN e4m3 overflows to inf above +-240 (unlike OCP's 448): clip first.
    return np.clip(np.asarray(x, np.float32), -240.0, 240.0).astype(
        ml_dtypes.float8_e4m3)


def _host_prep(inp, dist=True):
    """Edge-weight MLP + row-normalization + fp8 A slices + all linearity
    folds. Pure per-graph preprocessing (no time loop). B = A@A is built
    on-device; x/node multi-hop contributions use exact f32 A here."""
    f = np.float32
    bf = ml_dtypes.bfloat16
    row, col = np.asarray(inp["sparse_idx"])
    loc = np.asarray(inp["loc"], f)
    delta = loc[col] - loc[row]
    h1 = np.tanh(delta @ np.asarray(inp["Wk0"], f) + np.asarray(inp["bk0"], f))
    h2 = np.tanh(h1 @ np.asarray(inp["Wk1"], f) + np.asarray(inp["bk1"], f))
    ker = _softplus((h2 @ np.asarray(inp["Wk2"], f)
                     + np.asarray(inp["bk2"], f))[:, 0])
    geo = np.asarray(inp["geodesic"], f)
    w = ker * np.asarray(inp["angle_ratio"], f) * np.exp(-geo * geo)
    denom = np.zeros(N, f)
    np.add.at(denom, row, w)
    w = (w / (denom[row] + np.float32(1e-8))).astype(f)
    A = np.zeros((N, N), f)
    np.add.at(A, (row, col), w)

    # fp8 A slices: T-layout k-tiles (moving operand) and natural row-tiles
    # (B-build stationary): a8t[k, p, m] = A[m, k*128+p]*SA,
    # an8[j, p, i] = A[j*128+p, i]*SA
    a8t = _q8(A.T * SA).reshape(NT, P, N)
    an8 = _q8(A * SA).reshape(NT, P, N)

    Wfe = np.asarray(inp["W_fe"], f)      # (1, 16)
    bfe = np.asarray(inp["b_fe"], f)
    Wp = np.asarray(inp["W_proj"], f)     # (64, 1)
    bp = np.asarray(inp["b_proj"], f)
    node = np.asarray(inp["node_emb"], f)
    SC = [1.0, SA, SB]

    # encoder fold: z rows per hop k are [feat16 | node16 | x1 | h64].
    # x/node hop rows are computed with exact f32 A on the host (no SC).
    def enc_fold(W):
        out = W.shape[1]
        Wx = np.zeros((3, out), f)
        b_extra = np.zeros(out, f)
        Wh = np.zeros((64, 3 * out), f)
        for k in range(3):
            Wk = W[k * 97:(k + 1) * 97]
            Wf, Wxr, Whk = Wk[0:16], Wk[32:33], Wk[33:97]
            Wx[k] = Wxr[0] + Wfe[0] @ Wf
            b_extra += bfe @ Wf
            Wh[:, k * out:(k + 1) * out] = Whk / SC[k]
        return Wx, Wh, b_extra

    Wg_e = np.asarray(inp["Wg_e"], f)
    Wc_e = np.asarray(inp["Wc_e"], f)
    wgx, wge, bg_x = enc_fold(Wg_e)
    wcx, wce, bc_x = enc_fold(Wc_e)
    bg_e = np.asarray(inp["bg_e"], f) + bg_x
    bc_e = np.asarray(inp["bc_e"], f) + bc_x

    # node rhs rows: [node.T; (A node).T; (B node).T] exact f32; per-hop
    # weight blocks stacked in wgn/wcn rows 0-47; rows 48-50 hold the
    # folded x/Ax/Bx weights (rhs rows DMA'd per step)
    Anode = A @ node
    nodeT = np.concatenate([node.T, Anode.T, (A @ Anode).T], axis=0)
    wgn = np.zeros((51, 128), f)
    wcn = np.zeros((51, 64), f)
    for k in range(3):
        wgn[k * 16:(k + 1) * 16] = Wg_e[k * 97 + 16:k * 97 + 32]
        wcn[k * 16:(k + 1) * 16] = Wc_e[k * 97 + 16:k * 97 + 32]
    wgn[48:51] = wgx
    wcn[48:51] = wcx

    # decoder fold: z rows per hop k are [y1 | h64]
    Wg_d = np.asarray(inp["Wg_d"], f)
    Wc_d = np.asarray(inp["Wc_d"], f)

    def dec_fold(W):
        out = W.shape[1]
        Wh_plain = np.zeros((64, 3 * out), f)
        Wh_fold = np.zeros((64, 3 * out), f)
        Wy_h = np.zeros((64, 3 * out), f)
        b_extra = np.zeros(out, f)
        for k in range(3):
            Wk = W[k * 65:(k + 1) * 65]
            Wy, Wh = Wk[0:1], Wk[1:65]
            Wh_plain[:, k * out:(k + 1) * out] = Wh / SC[k]
            Wh_fold[:, k * out:(k + 1) * out] = (Wh + Wp @ Wy) / SC[k]
            Wy_h[:, k * out:(k + 1) * out] = (Wp @ Wy) / SC[k]
            b_extra += bp @ Wy
        return Wh_plain, Wh_fold, Wy_h, b_extra

    wgd0, wgdf, _, bgd_x = dec_fold(Wg_d)
    wcd0, _, wcdy, bcd_x = dec_fold(Wc_d)
    bg_d = np.asarray(inp["bg_d"], f)
    bc_d = np.asarray(inp["bc_d"], f)

    bias = np.zeros((64, 12), f)
    bias[:, 0] = bg_e[0:64]
    bias[:, 1] = bg_e[64:128]
    bias[:, 2] = bc_e
    bias[:, 3] = bg_d[0:64]
    bias[:, 4] = bg_d[64:128]
    bias[:, 5] = (bg_d + bgd_x)[0:64]
    bias[:, 6] = (bg_d + bgd_x)[64:128]
    bias[:, 7] = bc_d
    bias[:, 8] = bc_d + bcd_x
    bias[0, 9] = bp[0]

    shared = {
        "nodeT": np.ascontiguousarray(nodeT).astype(bf),
        "wge1": np.concatenate([wge[:, 0:128], wge[:, 128:256]]).astype(bf),
        "wge2": np.concatenate([wge[:, 256:384], wgn]).astype(bf),
        "wce1": np.concatenate([wce[:, 0:64], wce[:, 64:128]]).astype(bf),
        "wce2": np.concatenate([wce[:, 128:192], wcn]).astype(bf),
        "wgd01": np.concatenate([wgd0[:, 0:128], wgd0[:, 128:256]]).astype(bf),
        "wgd02": wgd0[:, 256:384].astype(bf),
        "wgdf1": np.concatenate([wgdf[:, 0:128], wgdf[:, 128:256]]).astype(bf),
        "wgdf2": wgdf[:, 256:384].astype(bf),
        "wcd01": np.concatenate([wcd0[:, 0:64], wcd0[:, 64:128]]).astype(bf),
        "wcd02": wcd0[:, 128:192].astype(bf),
        "wcdy1": np.concatenate([wcdy[:, 0:64], wcdy[:, 64:128]]).astype(bf),
        "wcdy2": wcdy[:, 128:192].astype(bf),
        "wproj": Wp.astype(bf), "bias": bias,
    }
    if dist:
        pass
    else:
        shared["afull"] = a8t
        shared["anfull"] = an8

    xs = np.asarray(inp["inputs"], f)[:, :, :, 0]    # (S, B, N)
    in_maps = []
    for b in range(NCORES):
        X = xs[:, b, :]                              # (S, N)
        AXt = X @ A.T                                # exact f32 (A@x).T rows
        BXt = AXt @ A.T
        xab = np.stack([X, AXt, BXt])                # (3, S, N)
        m = dict(shared)
        m["xab"] = np.ascontiguousarray(xab).astype(bf)
        if dist:
            m["aab"] = np.ascontiguousarray(
                np.concatenate([a8t[KSL * b:KSL * (b + 1)],
                                an8[KSL * b:KSL * (b + 1)]]))
        in_maps.append(m)
    return in_maps


_NC_CACHE = []


def kernel(**inputs):
    if not _NC_CACHE:
        _NC_CACHE.append(_build())
    nc = _NC_CACHE[0]
    in_maps = _host_prep(inputs)
    res = run_bass_kernel_spmd(nc, in_maps, core_ids=list(range(NCORES)))
    out = np.stack([res.results[b]["out"] for b in range(NCORES)], axis=1)
    return np.ascontiguousarray(out[..., None].astype(np.float32))
